# revision 19
# baseline (speedup 1.0000x reference)
"""Trainium2 Bass kernel: transformer decoder layer, 8 NeuronCores.

Problem: B=2, S=2048, D=1024, H=16 (HD=64), FF=4096, fp32 I/O, causal
attention (scores scaled by 1/HD), exact GELU, two LayerNorms.

Distribution (SPMD, identical program on all 8 cores; per-core data differs):
  - Head-parallel attention: core c computes Q/K/V projections and causal
    attention for heads {2c, 2c+1} over all B*S=4096 tokens. Activations are
    kept transposed ([dim, token]) so the whole attention chain needs no
    on-device transposes. Softmax runs in [key, query] layout: the sum of
    exp comes free from an extra ones-column appended to V; normalization
    uses a rank-1 PE broadcast of the reciprocal.
  - One AllToAll (1 MiB/core, bf16) converts head-sharded context into
    token-sharded context.
  - Token-parallel tail: core c computes Wo projection + residual + LN1 +
    FFN (full weights) + residual + LN2 for its 512 tokens.
  - Host stitches the 8 output shards (free).

Matmuls run in bf16 with fp32 PSUM accumulation; LayerNorm statistics and
residuals are fp32.
"""

import os
import sys

import numpy as np

sys.path.insert(0, "/opt/trn_rl_repo")

import ml_dtypes  # noqa: E402

from concourse import bacc, bass, mybir, tile  # noqa: E402

BF16 = mybir.dt.bfloat16
F32 = mybir.dt.float32
AF = mybir.ActivationFunctionType
ALU = mybir.AluOpType

FULL_CFG = dict(B=2, S=2048, D=1024, FF=4096, NCORES=8)


def _derive(cfg):
    B, S, D, FF, NC = cfg["B"], cfg["S"], cfg["D"], cfg["FF"], cfg["NCORES"]
    d = dict(cfg)
    d["HD"] = 64
    d["H"] = 2 * NC                      # heads total; 2 per core
    assert d["H"] * d["HD"] == D
    d["T"] = B * S                       # total tokens
    d["TOK"] = d["T"] // NC              # tokens owned per core after A2A
    d["TT"] = d["TOK"] // 128            # token tiles per core
    d["DC"] = D // 128                   # 128-row chunks of D
    d["NQ"] = d["T"] // 512              # 512-wide col chunks of all tokens
    d["QCH"] = S // 512                  # query chunks per batch sequence
    d["SC"] = S // 128                   # key chunks per batch sequence
    d["FFC"] = FF // 128
    d["DG"] = D // 512                   # 512-wide chunks of D
    assert d["TOK"] % 128 == 0 and S % 512 == 0 and D % 512 == 0
    return d


def build_nc(cfg):
    """Build the SPMD program (one Bacc graph, runs on all cores)."""
    c = _derive(cfg)
    B, S, D, FF = c["B"], c["S"], c["D"], c["FF"]
    T, TOK, TT, DC, NQ, QCH, SC, FFC, DG = (
        c["T"], c["TOK"], c["TT"], c["DC"], c["NQ"], c["QCH"], c["SC"],
        c["FFC"], c["DG"],
    )
    NCORES = c["NCORES"]

    nc = bacc.Bacc(
        "TRN2", target_bir_lowering=False, debug=False, num_devices=NCORES
    )

    def din(name, shape, dt=BF16):
        return nc.dram_tensor(name, list(shape), dt, kind="ExternalInput").ap()

    xT = din("xT", [D, T])
    wq = din("wq", [128, D])
    wk = din("wk", [128, D])
    wv = din("wv", [128, D])
    wo = din("wo", [D, D])
    w1p = din("w1p", [FFC, 128, D])
    w2p = din("w2p", [FFC, 128, D])
    xpbo = din("xpbo", [TOK, D], F32)
    bqk = din("bqk", [128, 2], F32)
    bvb = din("bvb", [128, 128], F32)
    b1c = din("b1c", [128, FFC], F32)
    b2b = din("b2b", [128, D])
    g1b = din("g1b", [128, D])
    be1b = din("be1b", [128, D])
    g2b = din("g2b", [128, D])
    be2b = din("be2b", [128, D])
    masks = din("masks", [128, 4 * 512])
    ident = din("ident", [128, 128], F32)
    onesb = din("onesb", [128, 64], mybir.dt.float32r)
    out = nc.dram_tensor("out", [TOK, D], F32, kind="ExternalOutput").ap()

    with tile.TileContext(nc) as tc:
        _emit(tc, locals(), c)
    nc.compile()
    return nc


def _emit(tc, t, c):
    nc = tc.nc
    B, S, D, FF = c["B"], c["S"], c["D"], c["FF"]
    T, TOK, TT, DC, NQ, QCH, SC, FFC, DG = (
        c["T"], c["TOK"], c["TT"], c["DC"], c["NQ"], c["QCH"], c["SC"],
        c["FFC"], c["DG"],
    )
    NCORES = c["NCORES"]
    xT, wq, wk, wv, wo, w1p, w2p = (
        t["xT"], t["wq"], t["wk"], t["wv"], t["wo"], t["w1p"], t["w2p"]
    )
    xpbo, bqk, bvb, b1c, b2b = t["xpbo"], t["bqk"], t["bvb"], t["b1c"], t["b2b"]
    g1b, be1b, g2b, be2b = t["g1b"], t["be1b"], t["g2b"], t["be2b"]
    masks, ident, onesb, out = t["masks"], t["ident"], t["onesb"], t["out"]

    from contextlib import ExitStack

    with ExitStack() as ctx:
        ep = ctx.enter_context  # helper

        # ---------- constant / persistent pools ----------
        cpool = ep(tc.tile_pool(name="consts", bufs=1))
        # urgent consts (needed in phases 1-2) -- small, issued first
        wq_sb = cpool.tile([128, D], BF16, name="wq_sb")
        nc.sync.dma_start(wq_sb[:, :], wq[:, :])
        wk_sb = cpool.tile([128, D], BF16, name="wk_sb")
        nc.sync.dma_start(wk_sb[:, :], wk[:, :])
        wv_sb = cpool.tile([128, D], BF16, name="wv_sb")
        nc.sync.dma_start(wv_sb[:, :], wv[:, :])
        bqk_sb = cpool.tile([128, 2], F32, name="bqk_sb")
        nc.sync.dma_start(bqk_sb[:, :], bqk[:, :])
        bvb_sb = cpool.tile([128, 128], F32, name="bvb_sb")
        nc.sync.dma_start(bvb_sb[:, :], bvb[:, :])
        mask_sb = cpool.tile([128, 4 * 512], BF16, name="mask_sb")
        nc.sync.dma_start(mask_sb[:, :], masks[:, :])
        ones_sb = cpool.tile([128, 64], mybir.dt.float32r, name="ones_sb")
        nc.sync.dma_start(ones_sb[:, :], onesb[:, :])
        eps_sb = cpool.tile([128, 1], F32, name="eps_sb")
        nc.vector.memset(eps_sb[:, :], 1e-5)
        # late consts: tiles now, DMAs deferred past the attention emission
        ident_sb = cpool.tile([128, 128], F32, name="ident_sb")
        b1_sb = cpool.tile([128, FFC], F32, name="b1_sb")
        b2b_sb = cpool.tile([128, D], BF16, name="b2b_sb")
        g1b_sb = cpool.tile([128, D], BF16, name="g1b_sb")
        be1b_sb = cpool.tile([128, D], BF16, name="be1b_sb")
        g2b_sb = cpool.tile([128, D], BF16, name="g2b_sb")
        be2b_sb = cpool.tile([128, D], BF16, name="be2b_sb")

        # resident weights: full W2 + Wo (DMAs emitted after phase 1 below)
        wpool = ep(tc.tile_pool(name="res_weights", bufs=1))
        w2_sb = wpool.tile([128, FFC * D], BF16, name="w2_sb")
        wo_sb = wpool.tile([128, NCORES * D], BF16, name="wo_sb")

        dpool = ep(tc.tile_pool(name="dram", bufs=1, space="DRAM"))
        # per-batch all-to-all: block d of pair b = this core's 2 heads of
        # context for batch b's d-th (TOK/2)-token chunk. Core r ends up
        # owning chunk r of batch 0 plus chunk r of batch 1.
        CH = TOK // B
        a2a_in_b = [dpool.tile([NCORES, 128, CH], BF16, name=f"a2a_in{b_}")
                    for b_ in range(B)]
        a2a_out_b = [dpool.tile([NCORES, 128, CH], BF16, name=f"a2a_out{b_}")
                     for b_ in range(B)]

        # ================= phases 1+2: QKV + attention =================
        with tc.tile_pool(name="attn_acts", bufs=1) as apool:
            qt_sb = apool.tile([128, T], BF16, name="qt_sb")   # Q^T, 2 heads
            kt_sb = apool.tile([128, T], BF16, name="kt_sb")   # K^T, 2 heads
            # V natural layout + a ones column per head: token tile tt, head
            # hh -> cols [tt*130 + hh*65 : +64] hold V, col +64 is 1.0
            v_sb = apool.tile([128, (T // 128) * 130], BF16, name="v_sb")
            v_sb4 = v_sb.rearrange("p (t h x) -> p t h x", h=2, x=65)
            nc.vector.memset(v_sb4[:, :, :, 64:65], 1.0)
            ctxh_sb = [
                apool.tile([64, T], BF16, name=f"ctxh{hh}_sb")
                for hh in range(2)
            ]

            # ----- phase 1: Q/K/V projections, one batch at a time -----
            with tc.tile_pool(name="xT_pool", bufs=2) as xpool, \
                 tc.tile_pool(name="qkv_psum", bufs=3, space="PSUM") as qkpsum, \
                 tc.tile_pool(name="v_psum", bufs=3, space="PSUM") as vpsum:
                bv3 = bvb_sb.rearrange("p (h e) -> p h e", h=2)
                for b in range(B):
                    xt_t = []
                    for dc in range(DC):
                        xt = xpool.tile([128, S], BF16, name=f"xt{b}_{dc}",
                                        tag=f"xt{dc}")
                        nc.sync.dma_start(
                            xt[:, :], xT[dc * 128:(dc + 1) * 128,
                                         b * S:(b + 1) * S])
                        xt_t.append(xt)
                    # Q^T and K^T: out [128 (2h*64), S]
                    for which, w_sb, bcol in ((0, wq_sb, 0), (1, wk_sb, 1)):
                        dst = qt_sb if which == 0 else kt_sb
                        for nq in range(S // 512):
                            ps = qkpsum.tile(
                                [128, 512], F32,
                                name=f"qk_ps{b}_{which}_{nq}", tag="qkps")
                            for dc in range(DC):
                                nc.tensor.matmul(
                                    ps[:, :],
                                    w_sb[:, dc * 128:(dc + 1) * 128],
                                    xt_t[dc][:, nq * 512:(nq + 1) * 512],
                                    start=(dc == 0), stop=(dc == DC - 1),
                                )
                            nc.vector.tensor_scalar(
                                dst[:, b * S + nq * 512:b * S + (nq + 1) * 512],
                                ps[:, :], bqk_sb[:, bcol:bcol + 1], None,
                                ALU.add,
                            )
                    # V natural: out [tok, 128 (2h*64)]
                    for tt in range(SC):
                        ps = vpsum.tile([128, 128], F32, name=f"v_ps{b}_{tt}",
                                        tag="vps")
                        for dc in range(DC):
                            nc.tensor.matmul(
                                ps[:, :],
                                xt_t[dc][:, tt * 128:(tt + 1) * 128],
                                wv_sb[:, dc * 128:(dc + 1) * 128],
                                start=(dc == 0), stop=(dc == DC - 1),
                            )
                        nc.vector.tensor_tensor(
                            v_sb4[:, b * SC + tt, :, 0:64],
                            ps.rearrange("p (h e) -> p h e", h=2),
                            bv3, ALU.add,
                        )

            # W2 + Wo prefetch: emitted after phase 1 so the xT loads win the
            # HBM bandwidth race at kernel start; stream in during attention.
            for g in range(FFC // 4):
                nc.sync.dma_start(
                    w2_sb.rearrange("p (f d) -> p f d", d=D)[:, 4 * g:4 * g + 4, :],
                    w2p.rearrange("f p d -> p f d")[:, 4 * g:4 * g + 4, :],
                )
            for p in range(NCORES):
                nc.sync.dma_start(
                    wo_sb[:, p * D:(p + 1) * D], wo[p * 128:(p + 1) * 128, :]
                )

            # ----- phase 2: causal attention (2 heads, both batches) -----
            with tc.tile_pool(name="exp_pool", bufs=6) as epool, \
                 tc.tile_pool(name="sc_psum", bufs=2, space="PSUM") as spsum, \
                 tc.tile_pool(name="ctx_psum", bufs=3, space="PSUM") as cpsum, \
                 tc.tile_pool(name="bc_psum", bufs=1, space="PSUM") as bpsum, \
                 tc.tile_pool(name="recip_pool", bufs=2) as rpool:
                for b in range(B):
                    for qi in range(QCH):
                        qlo = b * S + qi * 512
                        n_kc = 4 * (qi + 1)
                        for hh in range(2):
                            hof = 64 * hh
                            cps = cpsum.tile([65, 512], F32,
                                             name=f"ctx_{b}_{hh}_{qi}",
                                             tag="ctx")
                            for pj in range(n_kc // 2):
                                sc = spsum.tile([128, 1024], F32,
                                                name=f"sc_{b}_{hh}_{qi}_{pj}",
                                                tag="sc")
                                for j in range(2):
                                    kc = 2 * pj + j
                                    nc.tensor.matmul(
                                        sc[:, j * 512:(j + 1) * 512],
                                        kt_sb[hof:hof + 64,
                                              b * S + kc * 128:
                                              b * S + (kc + 1) * 128],
                                        qt_sb[hof:hof + 64, qlo:qlo + 512],
                                        start=True, stop=True,
                                    )
                                ex = epool.tile([128, 1024], BF16,
                                                name=f"ex_{b}_{hh}_{qi}_{pj}",
                                                tag="ex")
                                nc.scalar.activation(
                                    ex[:, :], sc[:, :], AF.Exp,
                                    scale=1.0 / 64.0)
                                for j in range(2):
                                    kc = 2 * pj + j
                                    exj = ex[:, j * 512:(j + 1) * 512]
                                    dg = kc - 4 * qi
                                    if dg >= 0:  # diagonal tile: causal mask
                                        nc.vector.tensor_mul(
                                            exj, exj,
                                            mask_sb[:, dg * 512:(dg + 1) * 512],
                                        )
                                    nc.tensor.matmul(
                                        cps[:, :],
                                        v_sb4[:, b * SC + kc, hh, :],
                                        exj,
                                        start=(kc == 0),
                                        stop=(kc == n_kc - 1),
                                    )
                            # rows 0..63 are ctx, row 64 is sum(exp).
                            # 1/sum as exp(-ln(sum)) on ScalarE: DVE's
                            # InstReciprocal costs ~6.5ns/elem regardless of
                            # partition count and sat on the critical chain.
                            rl = rpool.tile([65, 512], F32,
                                            name=f"rl_{b}_{hh}_{qi}", tag="rl")
                            nc.scalar.activation(
                                rl[64:65, :], cps[64:65, :], AF.Ln)
                            rc = rpool.tile([65, 512], mybir.dt.float32r,
                                            name=f"rc_{b}_{hh}_{qi}", tag="rc")
                            nc.scalar.activation(
                                rc[64:65, :], rl[64:65, :], AF.Exp, scale=-1.0)
                            bc = bpsum.tile([64, 512], F32,
                                            name=f"bc_{b}_{hh}_{qi}", tag="bc")
                            nc.tensor.matmul(
                                bc[:, :], ones_sb[64:65, :], rc[64:65, :],
                                start=True, stop=True,
                            )
                            bcs = rpool.tile([64, 512], F32,
                                             name=f"bcs_{b}_{hh}_{qi}",
                                             tag="bcs")
                            nc.vector.tensor_copy(bcs[:, :], bc[:, :])
                            nc.vector.tensor_mul(
                                ctxh_sb[hh][:, qlo:qlo + 512],
                                cps[0:64, :], bcs[:, :],
                            )
                        # ship this query block's chunks to the a2a buffer
                        for d in range(qi * 512 // CH, (qi * 512 + 512) // CH):
                            for hh in range(2):
                                nc.sync.dma_start(
                                    a2a_in_b[b][d, 64 * hh:64 * hh + 64, :],
                                    ctxh_sb[hh][:, b * S + d * CH:
                                                b * S + (d + 1) * CH],
                                )
                    # batch b fully shipped: launch its all-to-all (batch 0's
                    # overlaps batch 1's attention)
                    nc.gpsimd.collective_compute(
                        "AllToAll", ALU.bypass,
                        ins=[a2a_in_b[b].opt()], outs=[a2a_out_b[b].opt()],
                        replica_groups=[list(range(NCORES))],
                    )

        # late consts (needed from LN1 onwards; sync reaches these only once
        # the attention-phase queue drains)
        nc.sync.dma_start(ident_sb[:, :], ident[:, :])
        nc.sync.dma_start(b1_sb[:, :], b1c[:, :])
        nc.sync.dma_start(b2b_sb[:, :], b2b[:, :])
        nc.sync.dma_start(g1b_sb[:, :], g1b[:, :])
        nc.sync.dma_start(be1b_sb[:, :], be1b[:, :])
        nc.sync.dma_start(g2b_sb[:, :], g2b[:, :])
        nc.sync.dma_start(be2b_sb[:, :], be2b[:, :])

        # ---------- phases 4..8 ----------
        with tc.tile_pool(name="ln_pool", bufs=1) as ln_pool:
            ln1_t = [ln_pool.tile([128, D], F32, name=f"ln1_{t_}")
                     for t_ in range(TT)]
            ln1T_sb = ln_pool.tile([128, DC * TOK], BF16, name="ln1T_sb")

            # ----- phases 4+5: Wo projection, residual, LN1, transpose -----
            with tc.tile_pool(name="ctx_full", bufs=1) as cfpool, \
                 tc.tile_pool(name="xpbo_pool", bufs=1) as xppool, \
                 tc.tile_pool(name="resid_pool", bufs=1) as rspool, \
                 tc.tile_pool(name="wo_psum", bufs=2, space="PSUM") as wpsum, \
                 tc.tile_pool(name="tr_psum", bufs=3, space="PSUM") as tpsum, \
                 tc.tile_pool(name="stat_pool", bufs=4) as stpool:
                cf_sb = cfpool.tile([128, NCORES * TOK], BF16, name="cf_sb")
                for p in range(NCORES):
                    for b_ in range(B):
                        nc.sync.dma_start(
                            cf_sb[:, p * TOK + b_ * CH:p * TOK + (b_ + 1) * CH],
                            a2a_out_b[b_][p, :, :],
                        )
                xp_t = []
                for t_ in range(TT):
                    xp = xppool.tile([128, D], F32, name=f"xp{t_}",
                                     tag=f"xp{t_}")
                    nc.sync.dma_start(
                        xp[:, :], xpbo[t_ * 128:(t_ + 1) * 128, :])
                    xp_t.append(xp)
                for t_ in range(TT):
                    resid = rspool.tile([128, D], F32, name=f"res{t_}",
                                        tag=f"res{t_}")
                    for dg in range(DG):
                        ps = wpsum.tile([128, 512], F32,
                                        name=f"wo_ps{t_}_{dg}", tag="wops")
                        for p in range(NCORES):
                            nc.tensor.matmul(
                                ps[:, :],
                                cf_sb[:, p * TOK + t_ * 128:
                                      p * TOK + (t_ + 1) * 128],
                                wo_sb[:, p * D + dg * 512:
                                      p * D + (dg + 1) * 512],
                                start=(p == 0), stop=(p == NCORES - 1),
                            )
                        nc.vector.tensor_add(
                            resid[:, dg * 512:(dg + 1) * 512], ps[:, :],
                            xp_t[t_][:, dg * 512:(dg + 1) * 512],
                        )
                    _layer_norm(nc, stpool, t_, resid, ln1_t[t_], g1b_sb,
                                be1b_sb, D, eps_sb)
                    # transpose LN1 -> [D, tok] bf16 for the FFN
                    for dc in range(DC):
                        tp = tpsum.tile([128, 128], F32, name=f"tp{t_}_{dc}",
                                        tag="tp")
                        nc.tensor.transpose(
                            tp[:, :], ln1_t[t_][:, dc * 128:(dc + 1) * 128],
                            ident_sb[:, :],
                        )
                        nc.scalar.copy(
                            ln1T_sb[:, dc * TOK + t_ * 128:
                                    dc * TOK + (t_ + 1) * 128],
                            tp[:, :],
                        )

            # ----- phases 6-8: FFN + residual + LN2 -----
            with tc.tile_pool(name="h_pool", bufs=1) as hpool, \
                 tc.tile_pool(name="w1_pool", bufs=4) as w1pool, \
                 tc.tile_pool(name="ffn_psum", bufs=3, space="PSUM") as fpsum, \
                 tc.tile_pool(name="ff2_psum", bufs=2, space="PSUM") as f2psum, \
                 tc.tile_pool(name="out_pool", bufs=1) as opool, \
                 tc.tile_pool(name="stat2_pool", bufs=4) as st2pool:
                h_sb = hpool.tile([128, FFC * TOK], BF16, name="h_sb")
                for m in range(FFC):
                    w1t = w1pool.tile([128, D], BF16, name=f"w1t{m}",
                                      tag="w1t")
                    nc.sync.dma_start(w1t[:, :], w1p[m, :, :])
                    ph = fpsum.tile([128, TOK], F32, name=f"ff1_ps{m}",
                                    tag="ff1")
                    for dc in range(DC):
                        nc.tensor.matmul(
                            ph[:, :],
                            w1t[:, dc * 128:(dc + 1) * 128],
                            ln1T_sb[:, dc * TOK:(dc + 1) * TOK],
                            start=(dc == 0), stop=(dc == DC - 1),
                        )
                    nc.scalar.activation(
                        h_sb[:, m * TOK:(m + 1) * TOK], ph[:, :], AF.Gelu,
                        bias=b1_sb[:, m:m + 1],
                    )
                for t_ in range(TT):
                    res2 = opool.tile([128, D], F32, name=f"res2_{t_}",
                                      tag=f"res2_{t_}")
                    for dg in range(DG):
                        pf = f2psum.tile([128, 512], F32,
                                         name=f"ff2_ps{t_}_{dg}", tag="ff2")
                        for fc in range(FFC):
                            nc.tensor.matmul(
                                pf[:, :],
                                h_sb[:, fc * TOK + t_ * 128:
                                     fc * TOK + (t_ + 1) * 128],
                                w2_sb[:, fc * D + dg * 512:
                                      fc * D + (dg + 1) * 512],
                                start=(fc == 0), stop=(fc == FFC - 1),
                            )
                        nc.vector.tensor_add(
                            res2[:, dg * 512:(dg + 1) * 512], pf[:, :],
                            ln1_t[t_][:, dg * 512:(dg + 1) * 512],
                        )
                    nc.vector.tensor_add(res2[:, :], res2[:, :], b2b_sb[:, :])
                    # LN2 in place, then store
                    _layer_norm(nc, st2pool, t_, res2, res2, g2b_sb, be2b_sb,
                                D, eps_sb)
                    nc.sync.dma_start(
                        out[t_ * 128:(t_ + 1) * 128, :], res2[:, :])


def _layer_norm(nc, pool, t_, x_in, x_out, g_sb, be_sb, D, eps_sb):
    """x_out = (x_in - mean) * rsqrt(var + 1e-5) * g + be, stats over free axis."""
    ngrp = D // 512
    st6 = pool.tile([128, ngrp * 6], F32, name=f"st6_{t_}", tag="st6")
    for g in range(ngrp):
        nc.vector.bn_stats(
            st6[:, g * 6:(g + 1) * 6], x_in[:, g * 512:(g + 1) * 512]
        )
    mv = pool.tile([128, 2], F32, name=f"mv_{t_}", tag="mv")
    nc.vector.bn_aggr(mv[:, :], st6[:, :])
    std = pool.tile([128, 1], F32, name=f"std_{t_}", tag="std")
    nc.scalar.activation(std[:, :], mv[:, 1:2], AF.Sqrt, bias=eps_sb[:, :])
    rsd = pool.tile([128, 1], F32, name=f"rsd_{t_}", tag="rsd")
    nc.vector.reciprocal(rsd[:, :], std[:, :])
    nc.vector.tensor_scalar(
        x_out[:, :], x_in[:, :], mv[:, 0:1], rsd[:, :], ALU.subtract, ALU.mult
    )
    nc.vector.tensor_mul(x_out[:, :], x_out[:, :], g_sb[:, :])
    nc.vector.tensor_add(x_out[:, :], x_out[:, :], be_sb[:, :])


# ------------------------------------------------------------------
# host side
# ------------------------------------------------------------------

def prep_inputs(cfg, x, Wq, bq, Wk, bk, Wv, bv, Wo, bo, W1, b1, W2, b2,
                g1, be1, g2, be2):
    """Build the per-core input maps (list of dicts, one per core)."""
    c = _derive(cfg)
    D, FF, T, TOK, FFC, NCORES = (
        c["D"], c["FF"], c["T"], c["TOK"], c["FFC"], c["NCORES"]
    )
    B, S = c["B"], c["S"]
    CH = TOK // B
    bf = ml_dtypes.bfloat16
    f32 = np.float32

    def tobf(a):
        return np.ascontiguousarray(np.asarray(a, np.float32).astype(bf))

    xf = np.asarray(x, f32).reshape(T, D)
    xT = tobf(xf.T)
    # [H, D, HD] -> per-core [D, 128] -> prearranged [128, D]
    def prep_w(W, core):
        w = np.asarray(W, f32)[2 * core:2 * core + 2]        # [2, D, 64]
        w = w.transpose(1, 0, 2).reshape(D, 128)             # [D, 2*64]
        return tobf(w.reshape(D // 128, 128, 128).transpose(1, 0, 2)
                     .reshape(128, D))

    wo_b = tobf(np.asarray(Wo, f32))
    w1p = tobf(np.asarray(W1, f32).reshape(D // 128, 128, FFC, 128)
               .transpose(2, 1, 0, 3).reshape(FFC, 128, D))
    w2p = tobf(np.asarray(W2, f32).reshape(FFC, 128, D))
    b1c = np.ascontiguousarray(
        np.asarray(b1, f32).reshape(FFC, 128).T)
    b2bc = tobf(np.broadcast_to(np.asarray(b2, f32)[None, :], (128, D)))
    g1bc = tobf(np.broadcast_to(np.asarray(g1, f32)[None, :], (128, D)))
    be1bc = tobf(np.broadcast_to(np.asarray(be1, f32)[None, :], (128, D)))
    g2bc = tobf(np.broadcast_to(np.asarray(g2, f32)[None, :], (128, D)))
    be2bc = tobf(np.broadcast_to(np.asarray(be2, f32)[None, :], (128, D)))
    kk = np.arange(128)[:, None]
    qq = np.arange(512)[None, :]
    msk = np.stack([(kk + 128 * dg <= qq) for dg in range(4)], 0)
    masks = np.ascontiguousarray(
        msk.astype(bf).transpose(1, 0, 2).reshape(128, 4 * 512))
    identm = np.eye(128, dtype=f32)
    onesm = np.ones((128, 64), f32)

    bo_f = np.asarray(bo, f32)
    in_maps = []
    for core in range(NCORES):
        bq_c = np.asarray(bq, f32)[2 * core:2 * core + 2].reshape(128)
        bk_c = np.asarray(bk, f32)[2 * core:2 * core + 2].reshape(128)
        bv_c = np.asarray(bv, f32)[2 * core:2 * core + 2].reshape(128)
        in_maps.append(dict(
            xT=xT,
            wq=prep_w(Wq, core),
            wk=prep_w(Wk, core),
            wv=prep_w(Wv, core),
            wo=wo_b, w1p=w1p, w2p=w2p,
            xpbo=np.ascontiguousarray(
                np.concatenate(
                    [xf[b_ * S + core * CH:b_ * S + (core + 1) * CH]
                     for b_ in range(B)], 0) + bo_f[None, :]),
            bqk=np.ascontiguousarray(np.stack([bq_c, bk_c], 1)),
            bvb=np.ascontiguousarray(
                np.broadcast_to(bv_c[None, :], (128, 128))),
            b1c=b1c, b2b=b2bc, g1b=g1bc, be1b=be1bc, g2b=g2bc, be2b=be2bc,
            masks=masks, ident=identm, onesb=onesm,
        ))
    return in_maps


def assemble_output(cfg, results):
    c = _derive(cfg)
    B, S, D, TOK = c["B"], c["S"], c["D"], c["TOK"]
    CH = TOK // B
    full = np.empty((c["T"], D), np.float32)
    for core, res in enumerate(results):
        for b_ in range(B):
            full[b_ * S + core * CH:b_ * S + (core + 1) * CH] = \
                res["out"][b_ * CH:(b_ + 1) * CH]
    return full.reshape(B, S, D)


_NC_CACHE = {}


def _get_nc(cfg_key=None):
    key = tuple(sorted(FULL_CFG.items()))
    if key not in _NC_CACHE:
        _NC_CACHE[key] = build_nc(FULL_CFG)
    return _NC_CACHE[key]


def run_on_cores(in_maps, trace=False, **kw):
    from concourse.bass_utils import run_bass_kernel_spmd
    nc = _get_nc()
    return run_bass_kernel_spmd(
        nc, in_maps, core_ids=list(range(FULL_CFG["NCORES"])), trace=trace, **kw
    )


def kernel(**inputs):
    in_maps = prep_inputs(FULL_CFG, **inputs)
    res = run_on_cores(in_maps)
    return assemble_output(FULL_CFG, res.results)


# revision 20
# speedup vs baseline: 1.0580x; 1.0580x over previous
"""Trainium2 Bass kernel: transformer decoder layer, 8 NeuronCores.

Problem: B=2, S=2048, D=1024, H=16 (HD=64), FF=4096, fp32 I/O, causal
attention (scores scaled by 1/HD), exact GELU, two LayerNorms.

Distribution (SPMD, identical program on all 8 cores; per-core data differs):
  - Head-parallel attention: core c computes Q/K/V projections and causal
    attention for heads {2c, 2c+1} over all B*S=4096 tokens. Activations are
    kept transposed ([dim, token]) so the whole attention chain needs no
    on-device transposes. Softmax runs in [key, query] layout: the sum of
    exp comes free from an extra ones-column appended to V; normalization
    uses a rank-1 PE broadcast of the reciprocal.
  - One AllToAll (1 MiB/core, bf16) converts head-sharded context into
    token-sharded context.
  - Token-parallel tail: core c computes Wo projection + residual + LN1 +
    FFN (full weights) + residual + LN2 for its 512 tokens.
  - Host stitches the 8 output shards (free).

Matmuls run in bf16 with fp32 PSUM accumulation; LayerNorm statistics and
residuals are fp32.
"""

import os
import sys

import numpy as np

sys.path.insert(0, "/opt/trn_rl_repo")

import ml_dtypes  # noqa: E402

from concourse import bacc, bass, mybir, tile  # noqa: E402

BF16 = mybir.dt.bfloat16
F32 = mybir.dt.float32
AF = mybir.ActivationFunctionType
ALU = mybir.AluOpType

FULL_CFG = dict(B=2, S=2048, D=1024, FF=4096, NCORES=8)


def _derive(cfg):
    B, S, D, FF, NC = cfg["B"], cfg["S"], cfg["D"], cfg["FF"], cfg["NCORES"]
    d = dict(cfg)
    d["HD"] = 64
    d["H"] = 2 * NC                      # heads total; 2 per core
    assert d["H"] * d["HD"] == D
    d["T"] = B * S                       # total tokens
    d["TOK"] = d["T"] // NC              # tokens owned per core after A2A
    d["TT"] = d["TOK"] // 128            # token tiles per core
    d["DC"] = D // 128                   # 128-row chunks of D
    d["NQ"] = d["T"] // 512              # 512-wide col chunks of all tokens
    d["QCH"] = S // 512                  # query chunks per batch sequence
    d["SC"] = S // 128                   # key chunks per batch sequence
    d["FFC"] = FF // 128
    d["DG"] = D // 512                   # 512-wide chunks of D
    assert d["TOK"] % 128 == 0 and S % 512 == 0 and D % 512 == 0
    return d


def build_nc(cfg):
    """Build the SPMD program (one Bacc graph, runs on all cores)."""
    c = _derive(cfg)
    B, S, D, FF = c["B"], c["S"], c["D"], c["FF"]
    T, TOK, TT, DC, NQ, QCH, SC, FFC, DG = (
        c["T"], c["TOK"], c["TT"], c["DC"], c["NQ"], c["QCH"], c["SC"],
        c["FFC"], c["DG"],
    )
    NCORES = c["NCORES"]

    nc = bacc.Bacc(
        "TRN2", target_bir_lowering=False, debug=False, num_devices=NCORES
    )

    def din(name, shape, dt=BF16):
        return nc.dram_tensor(name, list(shape), dt, kind="ExternalInput").ap()

    xT = din("xT", [D, T])
    wq = din("wq", [128, D])
    wk = din("wk", [128, D])
    wv = din("wv", [128, D])
    wo = din("wo", [D, D])
    w1p = din("w1p", [FFC, 128, D])
    w2p = din("w2p", [FFC, 128, D])
    xpbo = din("xpbo", [TOK, D], F32)
    bqk = din("bqk", [128, 2], F32)
    bvb = din("bvb", [128, 128], F32)
    b1c = din("b1c", [128, FFC], F32)
    b2b = din("b2b", [128, D])
    g1b = din("g1b", [128, D])
    be1b = din("be1b", [128, D])
    g2b = din("g2b", [128, D])
    be2b = din("be2b", [128, D])
    masks = din("masks", [128, 4 * 512])
    ident = din("ident", [128, 128], F32)
    onesb = din("onesb", [128, 64], mybir.dt.float32r)
    out = nc.dram_tensor("out", [TOK, D], F32, kind="ExternalOutput").ap()

    with tile.TileContext(nc) as tc:
        _emit(tc, locals(), c)
    nc.compile()
    return nc


def _emit(tc, t, c):
    nc = tc.nc
    B, S, D, FF = c["B"], c["S"], c["D"], c["FF"]
    T, TOK, TT, DC, NQ, QCH, SC, FFC, DG = (
        c["T"], c["TOK"], c["TT"], c["DC"], c["NQ"], c["QCH"], c["SC"],
        c["FFC"], c["DG"],
    )
    NCORES = c["NCORES"]
    xT, wq, wk, wv, wo, w1p, w2p = (
        t["xT"], t["wq"], t["wk"], t["wv"], t["wo"], t["w1p"], t["w2p"]
    )
    xpbo, bqk, bvb, b1c, b2b = t["xpbo"], t["bqk"], t["bvb"], t["b1c"], t["b2b"]
    g1b, be1b, g2b, be2b = t["g1b"], t["be1b"], t["g2b"], t["be2b"]
    masks, ident, onesb, out = t["masks"], t["ident"], t["onesb"], t["out"]

    from contextlib import ExitStack

    with ExitStack() as ctx:
        ep = ctx.enter_context  # helper

        # ---------- constant / persistent pools ----------
        cpool = ep(tc.tile_pool(name="consts", bufs=1))
        # urgent consts (needed in phases 1-2) -- small, issued first
        wq_sb = cpool.tile([128, D], BF16, name="wq_sb")
        nc.sync.dma_start(wq_sb[:, :], wq[:, :])
        wk_sb = cpool.tile([128, D], BF16, name="wk_sb")
        nc.sync.dma_start(wk_sb[:, :], wk[:, :])
        wv_sb = cpool.tile([128, D], BF16, name="wv_sb")
        nc.sync.dma_start(wv_sb[:, :], wv[:, :])
        bqk_sb = cpool.tile([128, 2], F32, name="bqk_sb")
        nc.sync.dma_start(bqk_sb[:, :], bqk[:, :])
        bvb_sb = cpool.tile([128, 128], F32, name="bvb_sb")
        nc.sync.dma_start(bvb_sb[:, :], bvb[:, :])
        mask_sb = cpool.tile([128, 4 * 512], BF16, name="mask_sb")
        nc.sync.dma_start(mask_sb[:, :], masks[:, :])
        ones_sb = cpool.tile([128, 64], mybir.dt.float32r, name="ones_sb")
        nc.sync.dma_start(ones_sb[:, :], onesb[:, :])
        eps_sb = cpool.tile([128, 1], F32, name="eps_sb")
        nc.vector.memset(eps_sb[:, :], 1e-5)
        # late consts: tiles now, DMAs deferred past the attention emission
        ident_sb = cpool.tile([128, 128], F32, name="ident_sb")
        b1_sb = cpool.tile([128, FFC], F32, name="b1_sb")
        b2b_sb = cpool.tile([128, D], BF16, name="b2b_sb")
        g1b_sb = cpool.tile([128, D], BF16, name="g1b_sb")
        be1b_sb = cpool.tile([128, D], BF16, name="be1b_sb")
        g2b_sb = cpool.tile([128, D], BF16, name="g2b_sb")
        be2b_sb = cpool.tile([128, D], BF16, name="be2b_sb")

        # resident weights: full W2 + Wo (DMAs emitted after phase 1 below)
        wpool = ep(tc.tile_pool(name="res_weights", bufs=1))
        w2_sb = wpool.tile([128, FFC * D], BF16, name="w2_sb")
        wo_sb = wpool.tile([128, NCORES * D], BF16, name="wo_sb")

        dpool = ep(tc.tile_pool(name="dram", bufs=1, space="DRAM"))
        # per-batch all-to-all: block d of pair b = this core's 2 heads of
        # context for batch b's d-th (TOK/2)-token chunk. Core r ends up
        # owning chunk r of batch 0 plus chunk r of batch 1.
        CH = TOK // B
        a2a_in_b = [dpool.tile([NCORES, 128, CH], BF16, name=f"a2a_in{b_}")
                    for b_ in range(B)]
        a2a_out_b = [dpool.tile([NCORES, 128, CH], BF16, name=f"a2a_out{b_}")
                     for b_ in range(B)]

        # ================= phases 1+2: QKV + attention =================
        with tc.tile_pool(name="attn_acts", bufs=1) as apool:
            qt_sb = apool.tile([128, T], BF16, name="qt_sb")   # Q^T, 2 heads
            kt_sb = apool.tile([128, T], BF16, name="kt_sb")   # K^T, 2 heads
            # V natural layout + a ones column per head: token tile tt, head
            # hh -> cols [tt*130 + hh*65 : +64] hold V, col +64 is 1.0
            v_sb = apool.tile([128, (T // 128) * 130], BF16, name="v_sb")
            v_sb4 = v_sb.rearrange("p (t h x) -> p t h x", h=2, x=65)
            nc.vector.memset(v_sb4[:, :, :, 64:65], 1.0)
            ctxh_sb = [
                apool.tile([64, T], BF16, name=f"ctxh{hh}_sb")
                for hh in range(2)
            ]

            # ----- phase 1: Q/K/V projections, one batch at a time -----
            with tc.tile_pool(name="xT_pool", bufs=2) as xpool, \
                 tc.tile_pool(name="qkv_psum", bufs=3, space="PSUM") as qkpsum, \
                 tc.tile_pool(name="v_psum", bufs=3, space="PSUM") as vpsum:
                bv3 = bvb_sb.rearrange("p (h e) -> p h e", h=2)
                for b in range(B):
                    xt_t = []
                    for dc in range(DC):
                        xt = xpool.tile([128, S], BF16, name=f"xt{b}_{dc}",
                                        tag=f"xt{dc}")
                        nc.sync.dma_start(
                            xt[:, :], xT[dc * 128:(dc + 1) * 128,
                                         b * S:(b + 1) * S])
                        xt_t.append(xt)
                    # Q^T and K^T: out [128 (2h*64), S]
                    for which, w_sb, bcol in ((0, wq_sb, 0), (1, wk_sb, 1)):
                        dst = qt_sb if which == 0 else kt_sb
                        for nq in range(S // 512):
                            ps = qkpsum.tile(
                                [128, 512], F32,
                                name=f"qk_ps{b}_{which}_{nq}", tag="qkps")
                            for dc in range(DC):
                                nc.tensor.matmul(
                                    ps[:, :],
                                    w_sb[:, dc * 128:(dc + 1) * 128],
                                    xt_t[dc][:, nq * 512:(nq + 1) * 512],
                                    start=(dc == 0), stop=(dc == DC - 1),
                                )
                            nc.vector.tensor_scalar(
                                dst[:, b * S + nq * 512:b * S + (nq + 1) * 512],
                                ps[:, :], bqk_sb[:, bcol:bcol + 1], None,
                                ALU.add,
                            )
                    # V natural: out [tok, 128 (2h*64)]
                    for tt in range(SC):
                        ps = vpsum.tile([128, 128], F32, name=f"v_ps{b}_{tt}",
                                        tag="vps")
                        for dc in range(DC):
                            nc.tensor.matmul(
                                ps[:, :],
                                xt_t[dc][:, tt * 128:(tt + 1) * 128],
                                wv_sb[:, dc * 128:(dc + 1) * 128],
                                start=(dc == 0), stop=(dc == DC - 1),
                            )
                        nc.vector.tensor_tensor(
                            v_sb4[:, b * SC + tt, :, 0:64],
                            ps.rearrange("p (h e) -> p h e", h=2),
                            bv3, ALU.add,
                        )

            # W2 + Wo prefetch: emitted after phase 1 so the xT loads win the
            # HBM bandwidth race at kernel start; stream in during attention.
            for g in range(FFC // 4):
                nc.sync.dma_start(
                    w2_sb.rearrange("p (f d) -> p f d", d=D)[:, 4 * g:4 * g + 4, :],
                    w2p.rearrange("f p d -> p f d")[:, 4 * g:4 * g + 4, :],
                )
            for p in range(NCORES):
                nc.sync.dma_start(
                    wo_sb[:, p * D:(p + 1) * D], wo[p * 128:(p + 1) * 128, :]
                )

            # ----- phase 2: causal attention (2 heads, both batches) -----
            with tc.tile_pool(name="exp_pool", bufs=6) as epool, \
                 tc.tile_pool(name="sc_psum", bufs=2, space="PSUM") as spsum, \
                 tc.tile_pool(name="ctx_psum", bufs=3, space="PSUM") as cpsum, \
                 tc.tile_pool(name="bc_psum", bufs=1, space="PSUM") as bpsum, \
                 tc.tile_pool(name="recip_pool", bufs=2) as rpool:
                for b in range(B):
                    for qi in range(QCH):
                        qlo = b * S + qi * 512
                        n_kc = 4 * (qi + 1)
                        for hh in range(2):
                            hof = 64 * hh
                            cps = cpsum.tile([65, 512], F32,
                                             name=f"ctx_{b}_{hh}_{qi}",
                                             tag="ctx")
                            for pj in range(n_kc // 2):
                                sc = spsum.tile([128, 1024], F32,
                                                name=f"sc_{b}_{hh}_{qi}_{pj}",
                                                tag="sc")
                                for j in range(2):
                                    kc = 2 * pj + j
                                    nc.tensor.matmul(
                                        sc[:, j * 512:(j + 1) * 512],
                                        kt_sb[hof:hof + 64,
                                              b * S + kc * 128:
                                              b * S + (kc + 1) * 128],
                                        qt_sb[hof:hof + 64, qlo:qlo + 512],
                                        start=True, stop=True,
                                    )
                                ex = epool.tile([128, 1024], BF16,
                                                name=f"ex_{b}_{hh}_{qi}_{pj}",
                                                tag="ex")
                                nc.scalar.activation(
                                    ex[:, :], sc[:, :], AF.Exp,
                                    scale=1.0 / 64.0)
                                for j in range(2):
                                    kc = 2 * pj + j
                                    exj = ex[:, j * 512:(j + 1) * 512]
                                    dg = kc - 4 * qi
                                    if dg >= 0:  # diagonal tile: causal mask
                                        nc.vector.tensor_mul(
                                            exj, exj,
                                            mask_sb[:, dg * 512:(dg + 1) * 512],
                                        )
                                    nc.tensor.matmul(
                                        cps[:, :],
                                        v_sb4[:, b * SC + kc, hh, :],
                                        exj,
                                        start=(kc == 0),
                                        stop=(kc == n_kc - 1),
                                    )
                            # rows 0..63 are ctx, row 64 is sum(exp)
                            rc = rpool.tile([65, 512], mybir.dt.float32r,
                                            name=f"rc_{b}_{hh}_{qi}", tag="rc")
                            with nc.allow_low_precision(
                                    reason="softmax 1/sum in f32r feeds the "
                                           "f32r broadcast matmul"):
                                nc.vector.reciprocal(
                                    rc[64:65, :], cps[64:65, :])
                            bc = bpsum.tile([64, 512], F32,
                                            name=f"bc_{b}_{hh}_{qi}", tag="bc")
                            nc.tensor.matmul(
                                bc[:, :], ones_sb[64:65, :], rc[64:65, :],
                                start=True, stop=True,
                            )
                            bcs = rpool.tile([64, 512], F32,
                                             name=f"bcs_{b}_{hh}_{qi}",
                                             tag="bcs")
                            nc.vector.tensor_copy(bcs[:, :], bc[:, :])
                            nc.vector.tensor_mul(
                                ctxh_sb[hh][:, qlo:qlo + 512],
                                cps[0:64, :], bcs[:, :],
                            )
                        # ship this query block's chunks to the a2a buffer
                        for d in range(qi * 512 // CH, (qi * 512 + 512) // CH):
                            for hh in range(2):
                                nc.sync.dma_start(
                                    a2a_in_b[b][d, 64 * hh:64 * hh + 64, :],
                                    ctxh_sb[hh][:, b * S + d * CH:
                                                b * S + (d + 1) * CH],
                                )
                    # batch b fully shipped: launch its all-to-all (batch 0's
                    # overlaps batch 1's attention)
                    nc.gpsimd.collective_compute(
                        "AllToAll", ALU.bypass,
                        ins=[a2a_in_b[b].opt()], outs=[a2a_out_b[b].opt()],
                        replica_groups=[list(range(NCORES))],
                    )

        # late consts (needed from LN1 onwards; sync reaches these only once
        # the attention-phase queue drains)
        nc.sync.dma_start(ident_sb[:, :], ident[:, :])
        nc.sync.dma_start(b1_sb[:, :], b1c[:, :])
        nc.sync.dma_start(b2b_sb[:, :], b2b[:, :])
        nc.sync.dma_start(g1b_sb[:, :], g1b[:, :])
        nc.sync.dma_start(be1b_sb[:, :], be1b[:, :])
        nc.sync.dma_start(g2b_sb[:, :], g2b[:, :])
        nc.sync.dma_start(be2b_sb[:, :], be2b[:, :])

        # ---------- phases 4..8 ----------
        with tc.tile_pool(name="ln_pool", bufs=1) as ln_pool:
            ln1_t = [ln_pool.tile([128, D], F32, name=f"ln1_{t_}")
                     for t_ in range(TT)]
            ln1T_sb = ln_pool.tile([128, DC * TOK], BF16, name="ln1T_sb")

            # ----- phases 4+5: Wo projection, residual, LN1, transpose -----
            with tc.tile_pool(name="ctx_full", bufs=1) as cfpool, \
                 tc.tile_pool(name="xpbo_pool", bufs=1) as xppool, \
                 tc.tile_pool(name="resid_pool", bufs=1) as rspool, \
                 tc.tile_pool(name="wo_psum", bufs=2, space="PSUM") as wpsum, \
                 tc.tile_pool(name="tr_psum", bufs=3, space="PSUM") as tpsum, \
                 tc.tile_pool(name="stat_pool", bufs=4) as stpool:
                cf_sb = cfpool.tile([128, NCORES * TOK], BF16, name="cf_sb")
                for p in range(NCORES):
                    for b_ in range(B):
                        nc.sync.dma_start(
                            cf_sb[:, p * TOK + b_ * CH:p * TOK + (b_ + 1) * CH],
                            a2a_out_b[b_][p, :, :],
                        )
                xp_t = []
                for t_ in range(TT):
                    xp = xppool.tile([128, D], F32, name=f"xp{t_}",
                                     tag=f"xp{t_}")
                    nc.sync.dma_start(
                        xp[:, :], xpbo[t_ * 128:(t_ + 1) * 128, :])
                    xp_t.append(xp)
                for t_ in range(TT):
                    resid = rspool.tile([128, D], F32, name=f"res{t_}",
                                        tag=f"res{t_}")
                    for dg in range(DG):
                        ps = wpsum.tile([128, 512], F32,
                                        name=f"wo_ps{t_}_{dg}", tag="wops")
                        for p in range(NCORES):
                            nc.tensor.matmul(
                                ps[:, :],
                                cf_sb[:, p * TOK + t_ * 128:
                                      p * TOK + (t_ + 1) * 128],
                                wo_sb[:, p * D + dg * 512:
                                      p * D + (dg + 1) * 512],
                                start=(p == 0), stop=(p == NCORES - 1),
                            )
                        nc.vector.tensor_add(
                            resid[:, dg * 512:(dg + 1) * 512], ps[:, :],
                            xp_t[t_][:, dg * 512:(dg + 1) * 512],
                        )
                    _layer_norm(nc, stpool, t_, resid, ln1_t[t_], g1b_sb,
                                be1b_sb, D, eps_sb)
                    # transpose LN1 -> [D, tok] bf16 for the FFN
                    for dc in range(DC):
                        tp = tpsum.tile([128, 128], F32, name=f"tp{t_}_{dc}",
                                        tag="tp")
                        nc.tensor.transpose(
                            tp[:, :], ln1_t[t_][:, dc * 128:(dc + 1) * 128],
                            ident_sb[:, :],
                        )
                        nc.scalar.copy(
                            ln1T_sb[:, dc * TOK + t_ * 128:
                                    dc * TOK + (t_ + 1) * 128],
                            tp[:, :],
                        )

            # ----- phases 6-8: FFN + residual + LN2 -----
            with tc.tile_pool(name="h_pool", bufs=1) as hpool, \
                 tc.tile_pool(name="w1_pool", bufs=4) as w1pool, \
                 tc.tile_pool(name="ffn_psum", bufs=3, space="PSUM") as fpsum, \
                 tc.tile_pool(name="ff2_psum", bufs=2, space="PSUM") as f2psum, \
                 tc.tile_pool(name="out_pool", bufs=1) as opool, \
                 tc.tile_pool(name="stat2_pool", bufs=4) as st2pool:
                h_sb = hpool.tile([128, FFC * TOK], BF16, name="h_sb")
                for m in range(FFC):
                    w1t = w1pool.tile([128, D], BF16, name=f"w1t{m}",
                                      tag="w1t")
                    nc.sync.dma_start(w1t[:, :], w1p[m, :, :])
                    ph = fpsum.tile([128, TOK], F32, name=f"ff1_ps{m}",
                                    tag="ff1")
                    for dc in range(DC):
                        nc.tensor.matmul(
                            ph[:, :],
                            w1t[:, dc * 128:(dc + 1) * 128],
                            ln1T_sb[:, dc * TOK:(dc + 1) * TOK],
                            start=(dc == 0), stop=(dc == DC - 1),
                        )
                    nc.scalar.activation(
                        h_sb[:, m * TOK:(m + 1) * TOK], ph[:, :], AF.Gelu,
                        bias=b1_sb[:, m:m + 1],
                    )
                for t_ in range(TT):
                    res2 = opool.tile([128, D], F32, name=f"res2_{t_}",
                                      tag=f"res2_{t_}")
                    for dg in range(DG):
                        pf = f2psum.tile([128, 512], F32,
                                         name=f"ff2_ps{t_}_{dg}", tag="ff2")
                        for fc in range(FFC):
                            nc.tensor.matmul(
                                pf[:, :],
                                h_sb[:, fc * TOK + t_ * 128:
                                     fc * TOK + (t_ + 1) * 128],
                                w2_sb[:, fc * D + dg * 512:
                                      fc * D + (dg + 1) * 512],
                                start=(fc == 0), stop=(fc == FFC - 1),
                            )
                        nc.vector.tensor_add(
                            res2[:, dg * 512:(dg + 1) * 512], pf[:, :],
                            ln1_t[t_][:, dg * 512:(dg + 1) * 512],
                        )
                    nc.vector.tensor_add(res2[:, :], res2[:, :], b2b_sb[:, :])
                    # LN2 in place, then store
                    _layer_norm(nc, st2pool, t_, res2, res2, g2b_sb, be2b_sb,
                                D, eps_sb)
                    nc.sync.dma_start(
                        out[t_ * 128:(t_ + 1) * 128, :], res2[:, :])


def _layer_norm(nc, pool, t_, x_in, x_out, g_sb, be_sb, D, eps_sb):
    """x_out = (x_in - mean) * rsqrt(var + 1e-5) * g + be, stats over free axis."""
    ngrp = D // 512
    st6 = pool.tile([128, ngrp * 6], F32, name=f"st6_{t_}", tag="st6")
    for g in range(ngrp):
        nc.vector.bn_stats(
            st6[:, g * 6:(g + 1) * 6], x_in[:, g * 512:(g + 1) * 512]
        )
    mv = pool.tile([128, 2], F32, name=f"mv_{t_}", tag="mv")
    nc.vector.bn_aggr(mv[:, :], st6[:, :])
    std = pool.tile([128, 1], F32, name=f"std_{t_}", tag="std")
    nc.scalar.activation(std[:, :], mv[:, 1:2], AF.Sqrt, bias=eps_sb[:, :])
    rsd = pool.tile([128, 1], F32, name=f"rsd_{t_}", tag="rsd")
    nc.vector.reciprocal(rsd[:, :], std[:, :])
    nc.vector.tensor_scalar(
        x_out[:, :], x_in[:, :], mv[:, 0:1], rsd[:, :], ALU.subtract, ALU.mult
    )
    nc.vector.tensor_mul(x_out[:, :], x_out[:, :], g_sb[:, :])
    nc.vector.tensor_add(x_out[:, :], x_out[:, :], be_sb[:, :])


# ------------------------------------------------------------------
# host side
# ------------------------------------------------------------------

def prep_inputs(cfg, x, Wq, bq, Wk, bk, Wv, bv, Wo, bo, W1, b1, W2, b2,
                g1, be1, g2, be2):
    """Build the per-core input maps (list of dicts, one per core)."""
    c = _derive(cfg)
    D, FF, T, TOK, FFC, NCORES = (
        c["D"], c["FF"], c["T"], c["TOK"], c["FFC"], c["NCORES"]
    )
    B, S = c["B"], c["S"]
    CH = TOK // B
    bf = ml_dtypes.bfloat16
    f32 = np.float32

    def tobf(a):
        return np.ascontiguousarray(np.asarray(a, np.float32).astype(bf))

    xf = np.asarray(x, f32).reshape(T, D)
    xT = tobf(xf.T)
    # [H, D, HD] -> per-core [D, 128] -> prearranged [128, D]
    def prep_w(W, core):
        w = np.asarray(W, f32)[2 * core:2 * core + 2]        # [2, D, 64]
        w = w.transpose(1, 0, 2).reshape(D, 128)             # [D, 2*64]
        return tobf(w.reshape(D // 128, 128, 128).transpose(1, 0, 2)
                     .reshape(128, D))

    wo_b = tobf(np.asarray(Wo, f32))
    w1p = tobf(np.asarray(W1, f32).reshape(D // 128, 128, FFC, 128)
               .transpose(2, 1, 0, 3).reshape(FFC, 128, D))
    w2p = tobf(np.asarray(W2, f32).reshape(FFC, 128, D))
    b1c = np.ascontiguousarray(
        np.asarray(b1, f32).reshape(FFC, 128).T)
    b2bc = tobf(np.broadcast_to(np.asarray(b2, f32)[None, :], (128, D)))
    g1bc = tobf(np.broadcast_to(np.asarray(g1, f32)[None, :], (128, D)))
    be1bc = tobf(np.broadcast_to(np.asarray(be1, f32)[None, :], (128, D)))
    g2bc = tobf(np.broadcast_to(np.asarray(g2, f32)[None, :], (128, D)))
    be2bc = tobf(np.broadcast_to(np.asarray(be2, f32)[None, :], (128, D)))
    kk = np.arange(128)[:, None]
    qq = np.arange(512)[None, :]
    msk = np.stack([(kk + 128 * dg <= qq) for dg in range(4)], 0)
    masks = np.ascontiguousarray(
        msk.astype(bf).transpose(1, 0, 2).reshape(128, 4 * 512))
    identm = np.eye(128, dtype=f32)
    onesm = np.ones((128, 64), f32)

    bo_f = np.asarray(bo, f32)
    in_maps = []
    for core in range(NCORES):
        bq_c = np.asarray(bq, f32)[2 * core:2 * core + 2].reshape(128)
        bk_c = np.asarray(bk, f32)[2 * core:2 * core + 2].reshape(128)
        bv_c = np.asarray(bv, f32)[2 * core:2 * core + 2].reshape(128)
        in_maps.append(dict(
            xT=xT,
            wq=prep_w(Wq, core),
            wk=prep_w(Wk, core),
            wv=prep_w(Wv, core),
            wo=wo_b, w1p=w1p, w2p=w2p,
            xpbo=np.ascontiguousarray(
                np.concatenate(
                    [xf[b_ * S + core * CH:b_ * S + (core + 1) * CH]
                     for b_ in range(B)], 0) + bo_f[None, :]),
            bqk=np.ascontiguousarray(np.stack([bq_c, bk_c], 1)),
            bvb=np.ascontiguousarray(
                np.broadcast_to(bv_c[None, :], (128, 128))),
            b1c=b1c, b2b=b2bc, g1b=g1bc, be1b=be1bc, g2b=g2bc, be2b=be2bc,
            masks=masks, ident=identm, onesb=onesm,
        ))
    return in_maps


def assemble_output(cfg, results):
    c = _derive(cfg)
    B, S, D, TOK = c["B"], c["S"], c["D"], c["TOK"]
    CH = TOK // B
    full = np.empty((c["T"], D), np.float32)
    for core, res in enumerate(results):
        for b_ in range(B):
            full[b_ * S + core * CH:b_ * S + (core + 1) * CH] = \
                res["out"][b_ * CH:(b_ + 1) * CH]
    return full.reshape(B, S, D)


_NC_CACHE = {}


def _get_nc(cfg_key=None):
    key = tuple(sorted(FULL_CFG.items()))
    if key not in _NC_CACHE:
        _NC_CACHE[key] = build_nc(FULL_CFG)
    return _NC_CACHE[key]


def run_on_cores(in_maps, trace=False, **kw):
    from concourse.bass_utils import run_bass_kernel_spmd
    nc = _get_nc()
    return run_bass_kernel_spmd(
        nc, in_maps, core_ids=list(range(FULL_CFG["NCORES"])), trace=trace, **kw
    )


def kernel(**inputs):
    in_maps = prep_inputs(FULL_CFG, **inputs)
    res = run_on_cores(in_maps)
    return assemble_output(FULL_CFG, res.results)


# revision 21
# speedup vs baseline: 1.0580x; 1.0000x over previous
"""Trainium2 Bass kernel: transformer decoder layer, 8 NeuronCores.

Problem: B=2, S=2048, D=1024, H=16 (HD=64), FF=4096, fp32 I/O, causal
attention (scores scaled by 1/HD), exact GELU, two LayerNorms.

Distribution (SPMD, identical program on all 8 cores; per-core data differs):
  - Head-parallel attention: core c computes Q/K/V projections and causal
    attention for heads {2c, 2c+1} over all B*S=4096 tokens. Activations are
    kept transposed ([dim, token]) so the whole attention chain needs no
    on-device transposes. Softmax runs in [key, query] layout: the sum of
    exp comes free from an extra ones-column appended to V; normalization
    uses a rank-1 PE broadcast of the reciprocal.
  - One AllToAll (1 MiB/core, bf16) converts head-sharded context into
    token-sharded context.
  - Token-parallel tail: core c computes Wo projection + residual + LN1 +
    FFN (full weights) + residual + LN2 for its 512 tokens.
  - Host stitches the 8 output shards (free).

Matmuls run in bf16 with fp32 PSUM accumulation; LayerNorm statistics and
residuals are fp32.
"""

import os
import sys

import numpy as np

sys.path.insert(0, "/opt/trn_rl_repo")

import ml_dtypes  # noqa: E402

from concourse import bacc, bass, mybir, tile  # noqa: E402

BF16 = mybir.dt.bfloat16
F32 = mybir.dt.float32
AF = mybir.ActivationFunctionType
ALU = mybir.AluOpType

FULL_CFG = dict(B=2, S=2048, D=1024, FF=4096, NCORES=8)


def _derive(cfg):
    B, S, D, FF, NC = cfg["B"], cfg["S"], cfg["D"], cfg["FF"], cfg["NCORES"]
    d = dict(cfg)
    d["HD"] = 64
    d["H"] = 2 * NC                      # heads total; 2 per core
    assert d["H"] * d["HD"] == D
    d["T"] = B * S                       # total tokens
    d["TOK"] = d["T"] // NC              # tokens owned per core after A2A
    d["TT"] = d["TOK"] // 128            # token tiles per core
    d["DC"] = D // 128                   # 128-row chunks of D
    d["NQ"] = d["T"] // 512              # 512-wide col chunks of all tokens
    d["QCH"] = S // 512                  # query chunks per batch sequence
    d["SC"] = S // 128                   # key chunks per batch sequence
    d["FFC"] = FF // 128
    d["DG"] = D // 512                   # 512-wide chunks of D
    assert d["TOK"] % 128 == 0 and S % 512 == 0 and D % 512 == 0
    return d


def build_nc(cfg):
    """Build the SPMD program (one Bacc graph, runs on all cores)."""
    c = _derive(cfg)
    B, S, D, FF = c["B"], c["S"], c["D"], c["FF"]
    T, TOK, TT, DC, NQ, QCH, SC, FFC, DG = (
        c["T"], c["TOK"], c["TT"], c["DC"], c["NQ"], c["QCH"], c["SC"],
        c["FFC"], c["DG"],
    )
    NCORES = c["NCORES"]

    nc = bacc.Bacc(
        "TRN2", target_bir_lowering=False, debug=False, num_devices=NCORES
    )

    def din(name, shape, dt=BF16):
        return nc.dram_tensor(name, list(shape), dt, kind="ExternalInput").ap()

    xT = din("xT", [D, T])
    wq = din("wq", [128, D])
    wk = din("wk", [128, D])
    wv = din("wv", [128, D])
    wo = din("wo", [D, D])
    w1p = din("w1p", [FFC, 128, D])
    w2p = din("w2p", [FFC, 128, D])
    xpbo = din("xpbo", [TOK, D], F32)
    bqk = din("bqk", [128, 2], F32)
    bvb = din("bvb", [128, 128], F32)
    b1c = din("b1c", [128, FFC], F32)
    b2b = din("b2b", [128, D])
    g1b = din("g1b", [128, D])
    be1b = din("be1b", [128, D])
    g2b = din("g2b", [128, D])
    be2b = din("be2b", [128, D])
    masks = din("masks", [128, 4 * 512])
    ident = din("ident", [128, 128], F32)
    onesb = din("onesb", [128, 64], mybir.dt.float32r)
    out = nc.dram_tensor("out", [TOK, D], F32, kind="ExternalOutput").ap()

    with tile.TileContext(nc) as tc:
        _emit(tc, locals(), c)
    nc.compile()
    return nc


def _emit(tc, t, c):
    nc = tc.nc
    B, S, D, FF = c["B"], c["S"], c["D"], c["FF"]
    T, TOK, TT, DC, NQ, QCH, SC, FFC, DG = (
        c["T"], c["TOK"], c["TT"], c["DC"], c["NQ"], c["QCH"], c["SC"],
        c["FFC"], c["DG"],
    )
    NCORES = c["NCORES"]
    xT, wq, wk, wv, wo, w1p, w2p = (
        t["xT"], t["wq"], t["wk"], t["wv"], t["wo"], t["w1p"], t["w2p"]
    )
    xpbo, bqk, bvb, b1c, b2b = t["xpbo"], t["bqk"], t["bvb"], t["b1c"], t["b2b"]
    g1b, be1b, g2b, be2b = t["g1b"], t["be1b"], t["g2b"], t["be2b"]
    masks, ident, onesb, out = t["masks"], t["ident"], t["onesb"], t["out"]

    from contextlib import ExitStack

    with ExitStack() as ctx:
        ep = ctx.enter_context  # helper

        # ---------- constant / persistent pools ----------
        cpool = ep(tc.tile_pool(name="consts", bufs=1))
        # urgent consts (needed in phases 1-2) -- small, issued first
        wq_sb = cpool.tile([128, D], BF16, name="wq_sb")
        nc.sync.dma_start(wq_sb[:, :], wq[:, :])
        wk_sb = cpool.tile([128, D], BF16, name="wk_sb")
        nc.sync.dma_start(wk_sb[:, :], wk[:, :])
        wv_sb = cpool.tile([128, D], BF16, name="wv_sb")
        nc.sync.dma_start(wv_sb[:, :], wv[:, :])
        bqk_sb = cpool.tile([128, 2], F32, name="bqk_sb")
        nc.sync.dma_start(bqk_sb[:, :], bqk[:, :])
        bvb_sb = cpool.tile([128, 128], F32, name="bvb_sb")
        nc.sync.dma_start(bvb_sb[:, :], bvb[:, :])
        mask_sb = cpool.tile([128, 4 * 512], BF16, name="mask_sb")
        nc.sync.dma_start(mask_sb[:, :], masks[:, :])
        ones_sb = cpool.tile([128, 64], mybir.dt.float32r, name="ones_sb")
        nc.sync.dma_start(ones_sb[:, :], onesb[:, :])
        eps_sb = cpool.tile([128, 1], F32, name="eps_sb")
        nc.vector.memset(eps_sb[:, :], 1e-5)
        # late consts: tiles now, DMAs deferred past the attention emission
        ident_sb = cpool.tile([128, 128], F32, name="ident_sb")
        b1_sb = cpool.tile([128, FFC], F32, name="b1_sb")
        b2b_sb = cpool.tile([128, D], BF16, name="b2b_sb")
        g1b_sb = cpool.tile([128, D], BF16, name="g1b_sb")
        be1b_sb = cpool.tile([128, D], BF16, name="be1b_sb")
        g2b_sb = cpool.tile([128, D], BF16, name="g2b_sb")
        be2b_sb = cpool.tile([128, D], BF16, name="be2b_sb")

        # resident weights: full W2 + Wo (DMAs emitted after phase 1 below)
        wpool = ep(tc.tile_pool(name="res_weights", bufs=1))
        w2_sb = wpool.tile([128, FFC * D], BF16, name="w2_sb")
        wo_sb = wpool.tile([128, NCORES * D], BF16, name="wo_sb")

        dpool = ep(tc.tile_pool(name="dram", bufs=1, space="DRAM"))
        # per-batch all-to-all: block d of pair b = this core's 2 heads of
        # context for batch b's d-th (TOK/2)-token chunk. Core r ends up
        # owning chunk r of batch 0 plus chunk r of batch 1.
        CH = TOK // B
        a2a_in_b = [dpool.tile([NCORES, 128, CH], BF16, name=f"a2a_in{b_}")
                    for b_ in range(B)]
        a2a_out_b = [dpool.tile([NCORES, 128, CH], BF16, name=f"a2a_out{b_}")
                     for b_ in range(B)]

        # ================= phases 1+2: QKV + attention =================
        with tc.tile_pool(name="attn_acts", bufs=1) as apool:
            qt_sb = apool.tile([128, T], BF16, name="qt_sb")   # Q^T, 2 heads
            kt_sb = apool.tile([128, T], BF16, name="kt_sb")   # K^T, 2 heads
            # V natural layout + a ones column per head: token tile tt, head
            # hh -> cols [tt*130 + hh*65 : +64] hold V, col +64 is 1.0
            v_sb = apool.tile([128, (T // 128) * 130], BF16, name="v_sb")
            v_sb4 = v_sb.rearrange("p (t h x) -> p t h x", h=2, x=65)
            nc.vector.memset(v_sb4[:, :, :, 64:65], 1.0)
            ctxh_sb = [
                apool.tile([64, T], BF16, name=f"ctxh{hh}_sb")
                for hh in range(2)
            ]

            # ----- phase 1: Q/K/V projections, one batch at a time -----
            with tc.tile_pool(name="xT_pool", bufs=2) as xpool, \
                 tc.tile_pool(name="qkv_psum", bufs=3, space="PSUM") as qkpsum, \
                 tc.tile_pool(name="v_psum", bufs=3, space="PSUM") as vpsum:
                bv3 = bvb_sb.rearrange("p (h e) -> p h e", h=2)
                for b in range(B):
                    xt_t = []
                    for dc in range(DC):
                        xt = xpool.tile([128, S], BF16, name=f"xt{b}_{dc}",
                                        tag=f"xt{dc}")
                        nc.sync.dma_start(
                            xt[:, :], xT[dc * 128:(dc + 1) * 128,
                                         b * S:(b + 1) * S])
                        xt_t.append(xt)
                    # Q^T and K^T: out [128 (2h*64), S]
                    for which, w_sb, bcol in ((0, wq_sb, 0), (1, wk_sb, 1)):
                        dst = qt_sb if which == 0 else kt_sb
                        for nq in range(S // 512):
                            ps = qkpsum.tile(
                                [128, 512], F32,
                                name=f"qk_ps{b}_{which}_{nq}", tag="qkps")
                            for dc in range(DC):
                                nc.tensor.matmul(
                                    ps[:, :],
                                    w_sb[:, dc * 128:(dc + 1) * 128],
                                    xt_t[dc][:, nq * 512:(nq + 1) * 512],
                                    start=(dc == 0), stop=(dc == DC - 1),
                                )
                            nc.vector.tensor_scalar(
                                dst[:, b * S + nq * 512:b * S + (nq + 1) * 512],
                                ps[:, :], bqk_sb[:, bcol:bcol + 1], None,
                                ALU.add,
                            )
                    # V natural: out [tok, 128 (2h*64)]
                    for tt in range(SC):
                        ps = vpsum.tile([128, 128], F32, name=f"v_ps{b}_{tt}",
                                        tag="vps")
                        for dc in range(DC):
                            nc.tensor.matmul(
                                ps[:, :],
                                xt_t[dc][:, tt * 128:(tt + 1) * 128],
                                wv_sb[:, dc * 128:(dc + 1) * 128],
                                start=(dc == 0), stop=(dc == DC - 1),
                            )
                        nc.vector.tensor_tensor(
                            v_sb4[:, b * SC + tt, :, 0:64],
                            ps.rearrange("p (h e) -> p h e", h=2),
                            bv3, ALU.add,
                        )

            # W2 + Wo prefetch: emitted after phase 1 so the xT loads win the
            # HBM bandwidth race at kernel start; stream in during attention.
            for g in range(FFC // 4):
                nc.sync.dma_start(
                    w2_sb.rearrange("p (f d) -> p f d", d=D)[:, 4 * g:4 * g + 4, :],
                    w2p.rearrange("f p d -> p f d")[:, 4 * g:4 * g + 4, :],
                )
            for p in range(NCORES):
                nc.sync.dma_start(
                    wo_sb[:, p * D:(p + 1) * D], wo[p * 128:(p + 1) * 128, :]
                )

            # ----- phase 2: causal attention (2 heads, both batches) -----
            with tc.tile_pool(name="exp_pool", bufs=6) as epool, \
                 tc.tile_pool(name="sc_psum", bufs=2, space="PSUM") as spsum, \
                 tc.tile_pool(name="ctx_psum", bufs=3, space="PSUM") as cpsum, \
                 tc.tile_pool(name="bc_psum", bufs=1, space="PSUM") as bpsum, \
                 tc.tile_pool(name="recip_pool", bufs=2) as rpool:
                for b in range(B):
                    for qi in range(QCH):
                        qlo = b * S + qi * 512
                        n_kc = 4 * (qi + 1)
                        for hh in range(2):
                            hof = 64 * hh
                            cps = cpsum.tile([65, 512], F32,
                                             name=f"ctx_{b}_{hh}_{qi}",
                                             tag="ctx")
                            for pj in range(n_kc // 2):
                                # diagonal chunks are trimmed to the columns
                                # the causal mask keeps: chunk kc = 4*qi + d
                                # only covers q >= 128*d of this 512-q block
                                sc = spsum.tile([128, 1024], F32,
                                                name=f"sc_{b}_{hh}_{qi}_{pj}",
                                                tag="sc")
                                offs = []
                                for j in range(2):
                                    kc = 2 * pj + j
                                    dg = kc - 4 * qi
                                    qoff = max(0, 128 * dg)
                                    offs.append((j, kc, dg, qoff))
                                    nc.tensor.matmul(
                                        sc[:, j * 512 + qoff:(j + 1) * 512],
                                        kt_sb[hof:hof + 64,
                                              b * S + kc * 128:
                                              b * S + (kc + 1) * 128],
                                        qt_sb[hof:hof + 64,
                                              qlo + qoff:qlo + 512],
                                        start=True, stop=True,
                                    )
                                ex = epool.tile([128, 1024], BF16,
                                                name=f"ex_{b}_{hh}_{qi}_{pj}",
                                                tag="ex")
                                if offs[0][3] == 0 and offs[1][3] == 0:
                                    nc.scalar.activation(
                                        ex[:, :], sc[:, :], AF.Exp,
                                        scale=1.0 / 64.0)
                                else:
                                    for j, kc, dg, qoff in offs:
                                        nc.scalar.activation(
                                            ex[:, j * 512 + qoff:
                                               (j + 1) * 512],
                                            sc[:, j * 512 + qoff:
                                               (j + 1) * 512],
                                            AF.Exp, scale=1.0 / 64.0)
                                for j, kc, dg, qoff in offs:
                                    if dg >= 0:
                                        # triangular boundary strip only
                                        strip = ex[:, j * 512 + qoff:
                                                   j * 512 + qoff + 128]
                                        nc.vector.tensor_mul(
                                            strip, strip, mask_sb[:, 0:128])
                                    nc.tensor.matmul(
                                        cps[:, qoff:512],
                                        v_sb4[:, b * SC + kc, hh, :],
                                        ex[:, j * 512 + qoff:(j + 1) * 512],
                                        start=(kc == 0),
                                        stop=(kc == n_kc - 1),
                                    )
                            # rows 0..63 are ctx, row 64 is sum(exp)
                            rc = rpool.tile([65, 512], mybir.dt.float32r,
                                            name=f"rc_{b}_{hh}_{qi}", tag="rc")
                            with nc.allow_low_precision(
                                    reason="softmax 1/sum in f32r feeds the "
                                           "f32r broadcast matmul"):
                                nc.vector.reciprocal(
                                    rc[64:65, :], cps[64:65, :])
                            bc = bpsum.tile([64, 512], F32,
                                            name=f"bc_{b}_{hh}_{qi}", tag="bc")
                            nc.tensor.matmul(
                                bc[:, :], ones_sb[64:65, :], rc[64:65, :],
                                start=True, stop=True,
                            )
                            bcs = rpool.tile([64, 512], F32,
                                             name=f"bcs_{b}_{hh}_{qi}",
                                             tag="bcs")
                            nc.vector.tensor_copy(bcs[:, :], bc[:, :])
                            nc.vector.tensor_mul(
                                ctxh_sb[hh][:, qlo:qlo + 512],
                                cps[0:64, :], bcs[:, :],
                            )
                        # ship this query block's chunks to the a2a buffer
                        for d in range(qi * 512 // CH, (qi * 512 + 512) // CH):
                            for hh in range(2):
                                nc.sync.dma_start(
                                    a2a_in_b[b][d, 64 * hh:64 * hh + 64, :],
                                    ctxh_sb[hh][:, b * S + d * CH:
                                                b * S + (d + 1) * CH],
                                )
                    # batch b fully shipped: launch its all-to-all (batch 0's
                    # overlaps batch 1's attention)
                    nc.gpsimd.collective_compute(
                        "AllToAll", ALU.bypass,
                        ins=[a2a_in_b[b].opt()], outs=[a2a_out_b[b].opt()],
                        replica_groups=[list(range(NCORES))],
                    )

        # late consts (needed from LN1 onwards; sync reaches these only once
        # the attention-phase queue drains)
        nc.sync.dma_start(ident_sb[:, :], ident[:, :])
        nc.sync.dma_start(b1_sb[:, :], b1c[:, :])
        nc.sync.dma_start(b2b_sb[:, :], b2b[:, :])
        nc.sync.dma_start(g1b_sb[:, :], g1b[:, :])
        nc.sync.dma_start(be1b_sb[:, :], be1b[:, :])
        nc.sync.dma_start(g2b_sb[:, :], g2b[:, :])
        nc.sync.dma_start(be2b_sb[:, :], be2b[:, :])

        # ---------- phases 4..8 ----------
        with tc.tile_pool(name="ln_pool", bufs=1) as ln_pool:
            ln1_t = [ln_pool.tile([128, D], F32, name=f"ln1_{t_}")
                     for t_ in range(TT)]
            ln1T_sb = ln_pool.tile([128, DC * TOK], BF16, name="ln1T_sb")

            # ----- phases 4+5: Wo projection, residual, LN1, transpose -----
            with tc.tile_pool(name="ctx_full", bufs=1) as cfpool, \
                 tc.tile_pool(name="xpbo_pool", bufs=1) as xppool, \
                 tc.tile_pool(name="resid_pool", bufs=1) as rspool, \
                 tc.tile_pool(name="wo_psum", bufs=2, space="PSUM") as wpsum, \
                 tc.tile_pool(name="tr_psum", bufs=3, space="PSUM") as tpsum, \
                 tc.tile_pool(name="stat_pool", bufs=4) as stpool:
                cf_sb = cfpool.tile([128, NCORES * TOK], BF16, name="cf_sb")
                for p in range(NCORES):
                    for b_ in range(B):
                        nc.sync.dma_start(
                            cf_sb[:, p * TOK + b_ * CH:p * TOK + (b_ + 1) * CH],
                            a2a_out_b[b_][p, :, :],
                        )
                xp_t = []
                for t_ in range(TT):
                    xp = xppool.tile([128, D], F32, name=f"xp{t_}",
                                     tag=f"xp{t_}")
                    nc.sync.dma_start(
                        xp[:, :], xpbo[t_ * 128:(t_ + 1) * 128, :])
                    xp_t.append(xp)
                for t_ in range(TT):
                    resid = rspool.tile([128, D], F32, name=f"res{t_}",
                                        tag=f"res{t_}")
                    for dg in range(DG):
                        ps = wpsum.tile([128, 512], F32,
                                        name=f"wo_ps{t_}_{dg}", tag="wops")
                        for p in range(NCORES):
                            nc.tensor.matmul(
                                ps[:, :],
                                cf_sb[:, p * TOK + t_ * 128:
                                      p * TOK + (t_ + 1) * 128],
                                wo_sb[:, p * D + dg * 512:
                                      p * D + (dg + 1) * 512],
                                start=(p == 0), stop=(p == NCORES - 1),
                            )
                        nc.vector.tensor_add(
                            resid[:, dg * 512:(dg + 1) * 512], ps[:, :],
                            xp_t[t_][:, dg * 512:(dg + 1) * 512],
                        )
                    _layer_norm(nc, stpool, t_, resid, ln1_t[t_], g1b_sb,
                                be1b_sb, D, eps_sb)
                    # transpose LN1 -> [D, tok] bf16 for the FFN
                    for dc in range(DC):
                        tp = tpsum.tile([128, 128], F32, name=f"tp{t_}_{dc}",
                                        tag="tp")
                        nc.tensor.transpose(
                            tp[:, :], ln1_t[t_][:, dc * 128:(dc + 1) * 128],
                            ident_sb[:, :],
                        )
                        nc.scalar.copy(
                            ln1T_sb[:, dc * TOK + t_ * 128:
                                    dc * TOK + (t_ + 1) * 128],
                            tp[:, :],
                        )

            # ----- phases 6-8: FFN + residual + LN2 -----
            with tc.tile_pool(name="h_pool", bufs=1) as hpool, \
                 tc.tile_pool(name="w1_pool", bufs=4) as w1pool, \
                 tc.tile_pool(name="ffn_psum", bufs=3, space="PSUM") as fpsum, \
                 tc.tile_pool(name="ff2_psum", bufs=2, space="PSUM") as f2psum, \
                 tc.tile_pool(name="out_pool", bufs=1) as opool, \
                 tc.tile_pool(name="stat2_pool", bufs=4) as st2pool:
                h_sb = hpool.tile([128, FFC * TOK], BF16, name="h_sb")
                for m in range(FFC):
                    w1t = w1pool.tile([128, D], BF16, name=f"w1t{m}",
                                      tag="w1t")
                    nc.sync.dma_start(w1t[:, :], w1p[m, :, :])
                    ph = fpsum.tile([128, TOK], F32, name=f"ff1_ps{m}",
                                    tag="ff1")
                    for dc in range(DC):
                        nc.tensor.matmul(
                            ph[:, :],
                            w1t[:, dc * 128:(dc + 1) * 128],
                            ln1T_sb[:, dc * TOK:(dc + 1) * TOK],
                            start=(dc == 0), stop=(dc == DC - 1),
                        )
                    nc.scalar.activation(
                        h_sb[:, m * TOK:(m + 1) * TOK], ph[:, :], AF.Gelu,
                        bias=b1_sb[:, m:m + 1],
                    )
                for t_ in range(TT):
                    res2 = opool.tile([128, D], F32, name=f"res2_{t_}",
                                      tag=f"res2_{t_}")
                    for dg in range(DG):
                        pf = f2psum.tile([128, 512], F32,
                                         name=f"ff2_ps{t_}_{dg}", tag="ff2")
                        for fc in range(FFC):
                            nc.tensor.matmul(
                                pf[:, :],
                                h_sb[:, fc * TOK + t_ * 128:
                                     fc * TOK + (t_ + 1) * 128],
                                w2_sb[:, fc * D + dg * 512:
                                      fc * D + (dg + 1) * 512],
                                start=(fc == 0), stop=(fc == FFC - 1),
                            )
                        nc.vector.tensor_add(
                            res2[:, dg * 512:(dg + 1) * 512], pf[:, :],
                            ln1_t[t_][:, dg * 512:(dg + 1) * 512],
                        )
                    nc.vector.tensor_add(res2[:, :], res2[:, :], b2b_sb[:, :])
                    # LN2 in place, then store
                    _layer_norm(nc, st2pool, t_, res2, res2, g2b_sb, be2b_sb,
                                D, eps_sb)
                    nc.sync.dma_start(
                        out[t_ * 128:(t_ + 1) * 128, :], res2[:, :])


def _layer_norm(nc, pool, t_, x_in, x_out, g_sb, be_sb, D, eps_sb):
    """x_out = (x_in - mean) * rsqrt(var + 1e-5) * g + be, stats over free axis."""
    ngrp = D // 512
    st6 = pool.tile([128, ngrp * 6], F32, name=f"st6_{t_}", tag="st6")
    for g in range(ngrp):
        nc.vector.bn_stats(
            st6[:, g * 6:(g + 1) * 6], x_in[:, g * 512:(g + 1) * 512]
        )
    mv = pool.tile([128, 2], F32, name=f"mv_{t_}", tag="mv")
    nc.vector.bn_aggr(mv[:, :], st6[:, :])
    std = pool.tile([128, 1], F32, name=f"std_{t_}", tag="std")
    nc.scalar.activation(std[:, :], mv[:, 1:2], AF.Sqrt, bias=eps_sb[:, :])
    rsd = pool.tile([128, 1], F32, name=f"rsd_{t_}", tag="rsd")
    nc.vector.reciprocal(rsd[:, :], std[:, :])
    nc.vector.tensor_scalar(
        x_out[:, :], x_in[:, :], mv[:, 0:1], rsd[:, :], ALU.subtract, ALU.mult
    )
    nc.vector.tensor_mul(x_out[:, :], x_out[:, :], g_sb[:, :])
    nc.vector.tensor_add(x_out[:, :], x_out[:, :], be_sb[:, :])


# ------------------------------------------------------------------
# host side
# ------------------------------------------------------------------

def prep_inputs(cfg, x, Wq, bq, Wk, bk, Wv, bv, Wo, bo, W1, b1, W2, b2,
                g1, be1, g2, be2):
    """Build the per-core input maps (list of dicts, one per core)."""
    c = _derive(cfg)
    D, FF, T, TOK, FFC, NCORES = (
        c["D"], c["FF"], c["T"], c["TOK"], c["FFC"], c["NCORES"]
    )
    B, S = c["B"], c["S"]
    CH = TOK // B
    bf = ml_dtypes.bfloat16
    f32 = np.float32

    def tobf(a):
        return np.ascontiguousarray(np.asarray(a, np.float32).astype(bf))

    xf = np.asarray(x, f32).reshape(T, D)
    xT = tobf(xf.T)
    # [H, D, HD] -> per-core [D, 128] -> prearranged [128, D]
    def prep_w(W, core):
        w = np.asarray(W, f32)[2 * core:2 * core + 2]        # [2, D, 64]
        w = w.transpose(1, 0, 2).reshape(D, 128)             # [D, 2*64]
        return tobf(w.reshape(D // 128, 128, 128).transpose(1, 0, 2)
                     .reshape(128, D))

    wo_b = tobf(np.asarray(Wo, f32))
    w1p = tobf(np.asarray(W1, f32).reshape(D // 128, 128, FFC, 128)
               .transpose(2, 1, 0, 3).reshape(FFC, 128, D))
    w2p = tobf(np.asarray(W2, f32).reshape(FFC, 128, D))
    b1c = np.ascontiguousarray(
        np.asarray(b1, f32).reshape(FFC, 128).T)
    b2bc = tobf(np.broadcast_to(np.asarray(b2, f32)[None, :], (128, D)))
    g1bc = tobf(np.broadcast_to(np.asarray(g1, f32)[None, :], (128, D)))
    be1bc = tobf(np.broadcast_to(np.asarray(be1, f32)[None, :], (128, D)))
    g2bc = tobf(np.broadcast_to(np.asarray(g2, f32)[None, :], (128, D)))
    be2bc = tobf(np.broadcast_to(np.asarray(be2, f32)[None, :], (128, D)))
    kk = np.arange(128)[:, None]
    qq = np.arange(512)[None, :]
    msk = np.stack([(kk + 128 * dg <= qq) for dg in range(4)], 0)
    masks = np.ascontiguousarray(
        msk.astype(bf).transpose(1, 0, 2).reshape(128, 4 * 512))
    identm = np.eye(128, dtype=f32)
    onesm = np.ones((128, 64), f32)

    bo_f = np.asarray(bo, f32)
    in_maps = []
    for core in range(NCORES):
        bq_c = np.asarray(bq, f32)[2 * core:2 * core + 2].reshape(128)
        bk_c = np.asarray(bk, f32)[2 * core:2 * core + 2].reshape(128)
        bv_c = np.asarray(bv, f32)[2 * core:2 * core + 2].reshape(128)
        in_maps.append(dict(
            xT=xT,
            wq=prep_w(Wq, core),
            wk=prep_w(Wk, core),
            wv=prep_w(Wv, core),
            wo=wo_b, w1p=w1p, w2p=w2p,
            xpbo=np.ascontiguousarray(
                np.concatenate(
                    [xf[b_ * S + core * CH:b_ * S + (core + 1) * CH]
                     for b_ in range(B)], 0) + bo_f[None, :]),
            bqk=np.ascontiguousarray(np.stack([bq_c, bk_c], 1)),
            bvb=np.ascontiguousarray(
                np.broadcast_to(bv_c[None, :], (128, 128))),
            b1c=b1c, b2b=b2bc, g1b=g1bc, be1b=be1bc, g2b=g2bc, be2b=be2bc,
            masks=masks, ident=identm, onesb=onesm,
        ))
    return in_maps


def assemble_output(cfg, results):
    c = _derive(cfg)
    B, S, D, TOK = c["B"], c["S"], c["D"], c["TOK"]
    CH = TOK // B
    full = np.empty((c["T"], D), np.float32)
    for core, res in enumerate(results):
        for b_ in range(B):
            full[b_ * S + core * CH:b_ * S + (core + 1) * CH] = \
                res["out"][b_ * CH:(b_ + 1) * CH]
    return full.reshape(B, S, D)


_NC_CACHE = {}


def _get_nc(cfg_key=None):
    key = tuple(sorted(FULL_CFG.items()))
    if key not in _NC_CACHE:
        _NC_CACHE[key] = build_nc(FULL_CFG)
    return _NC_CACHE[key]


def run_on_cores(in_maps, trace=False, **kw):
    from concourse.bass_utils import run_bass_kernel_spmd
    nc = _get_nc()
    return run_bass_kernel_spmd(
        nc, in_maps, core_ids=list(range(FULL_CFG["NCORES"])), trace=trace, **kw
    )


def kernel(**inputs):
    in_maps = prep_inputs(FULL_CFG, **inputs)
    res = run_on_cores(in_maps)
    return assemble_output(FULL_CFG, res.results)
